# revision 5
# baseline (speedup 1.0000x reference)
"""Bass/Trainium2 kernel for nn_EvoBinarizedLayer.

Reference computation (P=16 populations, B=512, I=O=2048, all values 0/1):
    out[p,b,o] = sum_i x[p,b,i]*w0[p,i,o] + (1-x[p,b,i])*w1[p,i,o]

Strategy:
  - Shard population dim P across 8 cores (2 pops/core), embarrassingly parallel.
  - Cast x/w to fp8e4m3 on host (0/1 values are exact); compute notx = 1-x on
    device (ACT/DVE); accumulate x@w0 + notx@w1 into the same PSUM bank via a
    single K=4096 "concat" contraction -> one accumulation group, no bias pass.
  - fp8 DoubleRow matmuls (K=256 per MM) for 2x PE throughput.
  - PSUM f32 accumulation of 0/1 products is exact (max 4096 < 2^24), so the
    result is bit-exact vs the f32 reference.

Host-side work is layout only: slicing, transpose, dtype cast, and the final
gather. All arithmetic (notx, matmuls) happens on device.
"""

import os

import numpy as np
import ml_dtypes

from concourse import bacc, tile, mybir
from concourse.bass_utils import run_bass_kernel_spmd

P_TOT, B, I, O = 16, 512, 2048, 2048
N_CORES = 8
PPC = P_TOT // N_CORES  # pops per core = 2
PART = 128

FP8 = mybir.dt.float8e4
F32 = mybir.dt.float32
NP_FP8 = ml_dtypes.float8_e4m3


def build_nc(ppc=PPC, b=B, i_dim=I, o_dim=O, n_cores=N_CORES, use_dr=True):
    """Build + compile the per-core Bass program (SPMD: same program, 8 cores)."""
    kt = i_dim // PART          # k-subtiles per weight tensor (16)
    nb = o_dim // 512           # o-blocks (4)
    mb = b // PART              # b-subtiles (4)
    DR = mybir.MatmulPerfMode.DoubleRow if use_dr else None
    kstep = 2 if use_dr else 1

    nc = bacc.Bacc("TRN2", target_bir_lowering=False, debug=False,
                   num_devices=n_cores)

    xt_d = nc.dram_tensor("xt", [ppc, PART, kt, b], FP8, kind="ExternalInput")
    w0_d = nc.dram_tensor("w0", [ppc, nb, PART, kt, 512], FP8, kind="ExternalInput")
    w1_d = nc.dram_tensor("w1", [ppc, nb, PART, kt, 512], FP8, kind="ExternalInput")
    out_d = nc.dram_tensor("out", [ppc, b, o_dim], F32, kind="ExternalOutput")

    with tile.TileContext(nc) as tc:
        with (
            tc.tile_pool(name="warm", bufs=1) as warm,
            tc.tile_pool(name="xpool", bufs=2) as xpool,
            tc.tile_pool(name="wpool", bufs=8) as wpool,
            tc.tile_pool(name="opool", bufs=4) as opool,
            tc.tile_pool(name="pspool", bufs=4, space="PSUM") as pspool,
            tc.tile_pool(name="warmps", bufs=1, space="PSUM") as warmps,
        ):
            for pop in range(ppc):
                xt = xpool.tile([PART, kt, b], FP8, tag="xt")
                nxt = xpool.tile([PART, kt, b], FP8, tag="nxt")
                # x chunked on the scalar ring ahead of w1: the first matmul
                # needs only xt[:, 0:2, :], so a 256KB first chunk unblocks
                # the first LDWEIGHTS ~10us sooner than one 1MB transfer.
                xch = min(4, kt)
                for ch in range(0, kt, xch):
                    nc.scalar.dma_start(out=xt[:, ch:ch + xch, :],
                                        in_=xt_d.ap()[pop, :, ch:ch + xch, :])
                    # notx = 1 - x  ==  (x * -1) + 1, per chunk
                    nc.vector.tensor_scalar(
                        nxt[:, ch:ch + xch, :], xt[:, ch:ch + xch, :], -1.0, 1.0,
                        mybir.AluOpType.mult, mybir.AluOpType.add,
                    )
                for nbi in range(nb):
                    w0t = wpool.tile([PART, kt, 512], FP8, tag="w")
                    w1t = wpool.tile([PART, kt, 512], FP8, tag="w")
                    # w0 loads on the sync HWDGE ring, w1 on the scalar HWDGE
                    # ring (output stores go via gpsimd/SWDGE) so stores never
                    # block weight prefetch in a shared FIFO. Chunked k-wise so
                    # the first matmuls start before the whole block lands; the
                    # very first block uses finer chunks to cut the startup
                    # bubble before the first LDWEIGHTS.
                    wch = 2 if (pop == 0 and nbi == 0) else 4
                    for ch in range(0, kt, wch):
                        nc.sync.dma_start(
                            out=w0t[:, ch:ch + wch, :],
                            in_=w0_d.ap()[pop, nbi, :, ch:ch + wch, :])
                        nc.scalar.dma_start(
                            out=w1t[:, ch:ch + wch, :],
                            in_=w1_d.ap()[pop, nbi, :, ch:ch + wch, :])
                    for m in range(mb):
                        ps = pspool.tile([PART, 512], F32)
                        msl = slice(m * PART, (m + 1) * PART)
                        nk = kt // kstep
                        for kd in range(nk):
                            ksl = slice(kd * kstep, (kd + 1) * kstep)
                            nc.tensor.matmul(
                                ps[:], lhsT=xt[:, ksl, msl], rhs=w0t[:, ksl, :],
                                start=(kd == 0), stop=False, perf_mode=DR,
                            )
                        for kd in range(nk):
                            ksl = slice(kd * kstep, (kd + 1) * kstep)
                            nc.tensor.matmul(
                                ps[:], lhsT=nxt[:, ksl, msl], rhs=w1t[:, ksl, :],
                                start=False, stop=(kd == nk - 1), perf_mode=DR,
                            )
                        ot = opool.tile([PART, 512], F32)
                        nc.vector.tensor_copy(ot[:], ps[:])
                        nc.gpsimd.dma_start(
                            out=out_d.ap()[pop, msl, nbi * 512:(nbi + 1) * 512],
                            in_=ot[:],
                        )
    nc.compile()
    return nc


def build_nc_v3(ppc=PPC, b=B, i_dim=I, o_dim=O, n_cores=N_CORES):
    """v3: concat scheme (as v1) with stationary reuse.

    All weights for one population stay SBUF-resident (8MB fp8); the matmul
    loop is m -> half -> kd -> nb so one LDWEIGHTS serves 4 matmuls (one per
    o-block), cutting LDW traffic 4x and keeping the PE stream dense. PSUM
    holds 4 accumulating banks (one per o-block) per m-subtile.
    """
    kt = i_dim // PART
    nb = o_dim // 512
    mb = b // PART
    DR = mybir.MatmulPerfMode.DoubleRow
    nk = kt // 2

    nc = bacc.Bacc("TRN2", target_bir_lowering=False, debug=False,
                   num_devices=n_cores)

    xt_d = nc.dram_tensor("xt", [ppc, PART, kt, b], FP8, kind="ExternalInput")
    w0_d = nc.dram_tensor("w0", [ppc, nb, PART, kt, 512], FP8, kind="ExternalInput")
    w1_d = nc.dram_tensor("w1", [ppc, nb, PART, kt, 512], FP8, kind="ExternalInput")
    out_d = nc.dram_tensor("out", [ppc, b, o_dim], F32, kind="ExternalOutput")

    with tile.TileContext(nc) as tc:
        with (
            tc.tile_pool(name="xpool", bufs=2) as xpool,
            tc.tile_pool(name="wpool", bufs=2 * nb * 2) as wpool,
            tc.tile_pool(name="opool", bufs=6) as opool,
            tc.tile_pool(name="pspool", bufs=8, space="PSUM") as pspool,
        ):
            for pop in range(ppc):
                xt = xpool.tile([PART, kt, b], FP8, tag="xt")
                nxt = xpool.tile([PART, kt, b], FP8, tag="nxt")
                nc.gpsimd.dma_start(out=xt[:], in_=xt_d.ap()[pop])
                nc.vector.tensor_scalar(
                    nxt[:], xt[:], -1.0, 1.0,
                    mybir.AluOpType.mult, mybir.AluOpType.add,
                )
                # all weights for this pop, k-chunked so matmuls start early;
                # w0 on the sync HWDGE ring, w1 on the scalar HWDGE ring
                w0t = [wpool.tile([PART, kt, 512], FP8, tag="w",
                                  name=f"w0t_{pop}_{i}") for i in range(nb)]
                w1t = [wpool.tile([PART, kt, 512], FP8, tag="w",
                                  name=f"w1t_{pop}_{i}") for i in range(nb)]
                for ch in range(0, kt, 4):
                    for nbi in range(nb):
                        nc.sync.dma_start(
                            out=w0t[nbi][:, ch:ch + 4, :],
                            in_=w0_d.ap()[pop, nbi, :, ch:ch + 4, :])
                        nc.scalar.dma_start(
                            out=w1t[nbi][:, ch:ch + 4, :],
                            in_=w1_d.ap()[pop, nbi, :, ch:ch + 4, :])
                for m in range(mb):
                    msl = slice(m * PART, (m + 1) * PART)
                    pss = [pspool.tile([PART, 512], F32, tag="ps",
                                       name=f"ps_{pop}_{m}_{i}") for i in range(nb)]
                    for half, (xsrc, wt) in enumerate(((xt, w0t), (nxt, w1t))):
                        for kd in range(nk):
                            ksl = slice(2 * kd, 2 * kd + 2)
                            for nbi in range(nb):
                                nc.tensor.matmul(
                                    pss[nbi][:], lhsT=xsrc[:, ksl, msl],
                                    rhs=wt[nbi][:, ksl, :],
                                    start=(half == 0 and kd == 0),
                                    stop=(half == 1 and kd == nk - 1),
                                    perf_mode=DR,
                                )
                    for nbi in range(nb):
                        ot = opool.tile([PART, 512], F32)
                        nc.vector.tensor_copy(ot[:], pss[nbi][:])
                        nc.gpsimd.dma_start(
                            out=out_d.ap()[pop, msl, nbi * 512:(nbi + 1) * 512],
                            in_=ot[:],
                        )
    nc.compile()
    return nc


def build_nc_v4(ppc=PPC, b=B, i_dim=I, o_dim=O, n_cores=N_CORES):
    """v4: out = x@(w0-w1) + colsum(w1), wd built by DVE+gpsimd tensor_tensor.

    Halves the PE matmul stream vs the concat scheme (K=2048 instead of 4096).
    Per o-block: load w0/w1, bias = colsum(w1) via an all-ones DR matmul,
    wd = w0-w1 with the k-subtiles split between vector (11) and gpsimd (5)
    engines, main matmuls accumulate x@wd, and the DVE evacuation adds bias
    (tensor_tensor add against a bias tile copied from the bias PSUM bank).
    """
    kt = i_dim // PART
    nb = o_dim // 512
    mb = b // PART
    DR = mybir.MatmulPerfMode.DoubleRow
    nk = kt // 2
    # all subtract work on DVE: offloading 2 k-subtiles to gpsimd measured
    # 128.6us vs 128.0us all-DVE — the DVE's 23us of idle means it is not
    # strictly binding, and the gpsimd offload does not pay
    kdve = kt

    nc = bacc.Bacc("TRN2", target_bir_lowering=False, debug=False,
                   num_devices=n_cores)

    xt_d = nc.dram_tensor("xt", [ppc, PART, kt, b], FP8, kind="ExternalInput")
    w0_d = nc.dram_tensor("w0", [ppc, nb, PART, kt, 512], FP8, kind="ExternalInput")
    w1_d = nc.dram_tensor("w1", [ppc, nb, PART, kt, 512], FP8, kind="ExternalInput")
    out_d = nc.dram_tensor("out", [ppc, b, o_dim], F32, kind="ExternalOutput")

    with tile.TileContext(nc) as tc:
        with (
            tc.tile_pool(name="const", bufs=1) as const,
            tc.tile_pool(name="xpool", bufs=2) as xpool,
            tc.tile_pool(name="wsrc", bufs=6) as wsrc,
            tc.tile_pool(name="wdpool", bufs=4) as wdpool,
            tc.tile_pool(name="bpool", bufs=3) as bpool,
            tc.tile_pool(name="opool", bufs=4) as opool,
            tc.tile_pool(name="pspool", bufs=4, space="PSUM") as pspool,
            tc.tile_pool(name="psbias", bufs=2, space="PSUM") as psbias,
        ):
            ones = const.tile([PART, 2, PART], FP8)
            nc.vector.memset(ones[:], 1.0)
            xts = {}
            state = {}
            blocks = [(pop, nbi) for pop in range(ppc) for nbi in range(nb)]

            def prepare(pop, nbi):
                if nbi == 0:
                    xt = xpool.tile([PART, kt, b], FP8, tag="xt",
                                    name=f"xt_{pop}")
                    xch = min(4, kt)
                    for ch in range(0, kt, xch):
                        nc.scalar.dma_start(
                            out=xt[:, ch:ch + xch, :],
                            in_=xt_d.ap()[pop, :, ch:ch + xch, :])
                    xts[pop] = xt
                w0t = wsrc.tile([PART, kt, 512], FP8, tag="ws",
                                name=f"w0t_{pop}_{nbi}")
                w1t = wsrc.tile([PART, kt, 512], FP8, tag="ws",
                                name=f"w1t_{pop}_{nbi}")
                wch = 2 if (pop == 0 and nbi == 0) else 4
                for ch in range(0, kt, wch):
                    nc.sync.dma_start(
                        out=w1t[:, ch:ch + wch, :],
                        in_=w1_d.ap()[pop, nbi, :, ch:ch + wch, :])
                    nc.scalar.dma_start(
                        out=w0t[:, ch:ch + wch, :],
                        in_=w0_d.ap()[pop, nbi, :, ch:ch + wch, :])
                # bias = colsum(w1) (all rows of psb identical)
                psb = psbias.tile([PART, 512], F32, tag="psb")
                for kd in range(nk):
                    ksl = slice(2 * kd, 2 * kd + 2)
                    nc.tensor.matmul(
                        psb[:], lhsT=ones[:], rhs=w1t[:, ksl, :],
                        start=(kd == 0), stop=(kd == nk - 1), perf_mode=DR)
                bias_sb = bpool.tile([PART, 512], F32, tag="bias")
                nc.vector.tensor_copy(bias_sb[:], psb[:])
                # wd = w0 - w1 on DVE in fine k-chunks; emitted one block
                # AHEAD of the consuming matmuls (software pipeline) so these
                # sit before the previous block's evacuations in the DVE FIFO
                wd = wdpool.tile([PART, kt, 512], FP8, tag="wd")
                sch = max(1, kt // 8)
                for ch in range(0, kdve, sch):
                    nc.vector.tensor_tensor(
                        wd[:, ch:ch + sch, :], w0t[:, ch:ch + sch, :],
                        w1t[:, ch:ch + sch, :], mybir.AluOpType.subtract)
                if kdve < kt:
                    nc.gpsimd.tensor_tensor(
                        wd[:, kdve:, :], w0t[:, kdve:, :], w1t[:, kdve:, :],
                        mybir.AluOpType.subtract)
                state[(pop, nbi)] = (wd, bias_sb)

            def main(pop, nbi):
                wd, bias_sb = state.pop((pop, nbi))
                xt = xts[pop]
                for m in range(mb):
                    ps = pspool.tile([PART, 512], F32, tag="ps",
                                     name=f"ps_{pop}_{nbi}_{m}")
                    msl = slice(m * PART, (m + 1) * PART)
                    for kd in range(nk):
                        ksl = slice(2 * kd, 2 * kd + 2)
                        nc.tensor.matmul(
                            ps[:], lhsT=xt[:, ksl, msl], rhs=wd[:, ksl, :],
                            start=(kd == 0), stop=(kd == nk - 1), perf_mode=DR)
                    ot = opool.tile([PART, 512], F32, tag="ot",
                                    name=f"ot_{pop}_{nbi}_{m}")
                    nc.vector.tensor_tensor(
                        ot[:], ps[:], bias_sb[:], mybir.AluOpType.add)
                    nc.gpsimd.dma_start(
                        out=out_d.ap()[pop, msl, nbi * 512:(nbi + 1) * 512],
                        in_=ot[:])

            for i in range(len(blocks) + 1):
                if i < len(blocks):
                    prepare(*blocks[i])
                if i > 0:
                    main(*blocks[i - 1])
    nc.compile()
    return nc


def build_nc_v5(ppc=PPC, b=B, i_dim=I, o_dim=O, n_cores=N_CORES,
                kdve=6, sub_ch=4):
    """v5: v4 algebra (out = x@(w0-w1) + colsum(w1)) rebalanced across engines.

    Baseline v4 bottleneck was the DVE: 99.7us busy (wd subtract 78us + evac
    22us) vs PE 69us warm floor. Changes:
      - wd subtract split DVE (k-subtiles [0:kdve)) / gpsimd ([kdve:kt)) —
        Pool runs 1.2GHz vs DVE 0.96GHz.
      - PSUM evac (ps + bias) split DVE (m=0,2) / gpsimd (m=1,3), emitting
        f16 directly (exact: outputs are integers <= 2048 = 2^11).
      - bias psum->SBUF copies on the ACT engine (otherwise idle).
      - stores on the sync HWDGE ring (f16, half the bytes); w0+x on scalar,
        w1 on sync; gpsimd dispatches no DMA.
      - software pipeline: loads 2 blocks ahead, bias+subtract 1 block ahead
        of the main matmul stream so the PE never stalls and HAM stays warm;
        dummy warm-up matmuls cover the initial DMA latency.
    """
    kt = i_dim // PART
    nb = o_dim // 512
    mb = b // PART
    DR = mybir.MatmulPerfMode.DoubleRow
    nk = kt // 2
    F16 = mybir.dt.float16

    nc = bacc.Bacc("TRN2", target_bir_lowering=False, debug=False,
                   num_devices=n_cores)

    xt_d = nc.dram_tensor("xt", [ppc, PART, kt, b], FP8, kind="ExternalInput")
    w0_d = nc.dram_tensor("w0", [ppc, nb, PART, kt, 512], FP8, kind="ExternalInput")
    w1_d = nc.dram_tensor("w1", [ppc, nb, PART, kt, 512], FP8, kind="ExternalInput")
    out_d = nc.dram_tensor("out", [ppc, b, o_dim], F16, kind="ExternalOutput")

    with tile.TileContext(nc) as tc:
        with (
            tc.tile_pool(name="const", bufs=1) as const,
            tc.tile_pool(name="xpool", bufs=2) as xpool,
            tc.tile_pool(name="wsrc", bufs=6) as wsrc,
            tc.tile_pool(name="wdpool", bufs=3) as wdpool,
            tc.tile_pool(name="bpool", bufs=3) as bpool,
            tc.tile_pool(name="opool", bufs=8) as opool,
            tc.tile_pool(name="pspool", bufs=4, space="PSUM") as pspool,
            tc.tile_pool(name="psbias", bufs=2, space="PSUM") as psbias,
            tc.tile_pool(name="pswarm", bufs=1, space="PSUM") as pswarm,
        ):
            ones = const.tile([PART, 2, PART], FP8)
            nc.vector.memset(ones[:], 1.0)
            # HAM warm-up: ~8 N=512 matmuls on garbage keep the PE busy
            # through the cold window while the first weight DMAs land.
            warm = pswarm.tile([PART, 512], F32)
            wsrc_warm = const.tile([PART, 2, 512], FP8)
            nc.vector.memset(wsrc_warm[:], 1.0)
            for wi in range(8):
                nc.tensor.matmul(warm[:], lhsT=ones[:], rhs=wsrc_warm[:],
                                 start=(wi == 0), stop=(wi == 7), perf_mode=DR)

            xts = {}
            w0ts = {}
            w1ts = {}
            wds = {}
            biases = {}
            blocks = [(pop, nbi) for pop in range(ppc) for nbi in range(nb)]

            def load(i):
                pop, nbi = blocks[i]
                if nbi == 0:
                    xt = xpool.tile([PART, kt, b], FP8, tag="xt",
                                    name=f"xt_{pop}")
                    for ch in range(0, kt, 8):
                        nc.scalar.dma_start(
                            out=xt[:, ch:ch + 8, :],
                            in_=xt_d.ap()[pop, :, ch:ch + 8, :])
                    xts[pop] = xt
                w0t = wsrc.tile([PART, kt, 512], FP8, tag="ws",
                                name=f"w0t_{i}")
                w1t = wsrc.tile([PART, kt, 512], FP8, tag="ws",
                                name=f"w1t_{i}")
                wch = 2 if i == 0 else 4
                for ch in range(0, kt, wch):
                    nc.sync.dma_start(
                        out=w1t[:, ch:ch + wch, :],
                        in_=w1_d.ap()[pop, nbi, :, ch:ch + wch, :])
                    nc.scalar.dma_start(
                        out=w0t[:, ch:ch + wch, :],
                        in_=w0_d.ap()[pop, nbi, :, ch:ch + wch, :])
                w0ts[i], w1ts[i] = w0t, w1t

            def prep(i):
                w0t, w1t = w0ts[i], w1ts.pop(i)
                # bias = colsum(w1) via all-ones DR matmul chain
                psb = psbias.tile([PART, 512], F32, tag="psb")
                for kd in range(nk):
                    ksl = slice(2 * kd, 2 * kd + 2)
                    nc.tensor.matmul(
                        psb[:], lhsT=ones[:], rhs=w1t[:, ksl, :],
                        start=(kd == 0), stop=(kd == nk - 1), perf_mode=DR)
                bias_sb = bpool.tile([PART, 512], F32, tag="bias")
                nc.scalar.activation(bias_sb[:], psb[:],
                                     mybir.ActivationFunctionType.Copy)
                biases[i] = bias_sb
                # wd = w0 - w1, split DVE / gpsimd
                wd = wdpool.tile([PART, kt, 512], FP8, tag="wd")
                for ch in range(0, kdve, sub_ch):
                    ce = min(ch + sub_ch, kdve)
                    nc.vector.tensor_tensor(
                        wd[:, ch:ce, :], w0t[:, ch:ce, :],
                        w1t[:, ch:ce, :], mybir.AluOpType.subtract)
                for ch in range(kdve, kt, sub_ch):
                    ce = min(ch + sub_ch, kt)
                    nc.gpsimd.tensor_tensor(
                        wd[:, ch:ce, :], w0t[:, ch:ce, :],
                        w1t[:, ch:ce, :], mybir.AluOpType.subtract)
                wds[i] = wd
                w0ts.pop(i)

            def main(i):
                pop, nbi = blocks[i]
                wd = wds.pop(i)
                bias_sb = biases.pop(i)
                xt = xts[pop]
                for m in range(mb):
                    ps = pspool.tile([PART, 512], F32, tag="ps",
                                     name=f"ps_{i}_{m}")
                    msl = slice(m * PART, (m + 1) * PART)
                    for kd in range(nk):
                        ksl = slice(2 * kd, 2 * kd + 2)
                        nc.tensor.matmul(
                            ps[:], lhsT=xt[:, ksl, msl], rhs=wd[:, ksl, :],
                            start=(kd == 0), stop=(kd == nk - 1), perf_mode=DR)
                    ot = opool.tile([PART, 512], F16, tag="ot",
                                    name=f"ot_{i}_{m}")
                    # gpsimd cannot read PSUM -> evac is DVE-only
                    nc.vector.tensor_tensor(
                        ot[:], ps[:], bias_sb[:], mybir.AluOpType.add)
                    nc.sync.dma_start(
                        out=out_d.ap()[pop, msl, nbi * 512:(nbi + 1) * 512],
                        in_=ot[:])

            load(0)
            load(1)
            prep(0)
            for i in range(len(blocks)):
                if i + 2 < len(blocks):
                    load(i + 2)
                if i + 1 < len(blocks):
                    prep(i + 1)
                main(i)
    nc.compile()
    return nc


def build_nc_v2(ppc=PPC, b=B, i_dim=I, o_dim=O, n_cores=N_CORES):
    """v2: algebraic rewrite out = x@(w0-w1) + colsum(w1).

    The w1 input tensor holds -w1 (sign applied during the host fp8 cast;
    walrus rejects cce_op=subtract but accepts add):
    - wd = w0 + (-w1) computed by the gpsimd DMA inline ALU (accum_op=add)
      while loading w0 — zero compute-engine cost.
    - colsum(-w1) = -bias via an all-ones stationary matmul against the tile
      while it still holds -w1, once per o-block.
    - main pass: psum = x @ wd, half the PE work of v1; evacuated as
      psum - (-bias) with a DVE tensor_tensor subtract.
    All values stay exact: x in {0,1}, wd in {-1,0,1} (fp8 exact), bias and
    accumulation in f32 (integers < 2^24).
    """
    kt = i_dim // PART
    nb = o_dim // 512
    mb = b // PART
    DR = mybir.MatmulPerfMode.DoubleRow
    nk = kt // 2

    nc = bacc.Bacc("TRN2", target_bir_lowering=False, debug=False,
                   num_devices=n_cores)

    xt_d = nc.dram_tensor("xt", [ppc, PART, kt, b], FP8, kind="ExternalInput")
    w0_d = nc.dram_tensor("w0", [ppc, nb, PART, kt, 512], FP8, kind="ExternalInput")
    w1_d = nc.dram_tensor("w1", [ppc, nb, PART, kt, 512], FP8, kind="ExternalInput")
    out_d = nc.dram_tensor("out", [ppc, b, o_dim], F32, kind="ExternalOutput")

    with tile.TileContext(nc) as tc:
        with (
            tc.tile_pool(name="const", bufs=1) as const,
            tc.tile_pool(name="xpool", bufs=2) as xpool,
            tc.tile_pool(name="wpool", bufs=4) as wpool,
            tc.tile_pool(name="bpool", bufs=2) as bpool,
            tc.tile_pool(name="opool", bufs=4) as opool,
            tc.tile_pool(name="pspool", bufs=4, space="PSUM") as pspool,
            tc.tile_pool(name="psbias", bufs=2, space="PSUM") as psbias,
        ):
            ones = const.tile([PART, 2, PART], FP8)
            nc.vector.memset(ones[:], 1.0)
            for pop in range(ppc):
                xt = xpool.tile([PART, kt, b], FP8, tag="xt")
                nc.scalar.dma_start(out=xt[:], in_=xt_d.ap()[pop])
                for nbi in range(nb):
                    # 544-wide rows (512 data + 32 pad): keeps every SBUF write
                    # run at 512B so the accum DMA's RMW ucode accepts it (runs
                    # >512B crash the exec unit), and stops the AP optimizer
                    # from merging rows into one big run.
                    wdp = wpool.tile([PART, kt, 544], FP8, tag="w")
                    wd = wdp[:, :, :512]
                    # 1) load -w1 (sync HWDGE ring)
                    wch = min(8, kt)
                    for ch in range(0, kt, wch):
                        nc.sync.dma_start(
                            out=wd[:, ch:ch + wch, :],
                            in_=w1_d.ap()[pop, nbi, :, ch:ch + wch, :])
                    # 2) -bias = colsum(-w1) while the tile still holds -w1
                    psb = psbias.tile([PART, 512], F32)
                    for kd in range(nk):
                        ksl = slice(2 * kd, 2 * kd + 2)
                        nc.tensor.matmul(
                            psb[:], lhsT=ones[:], rhs=wd[:, ksl, :],
                            start=(kd == 0), stop=(kd == nk - 1), perf_mode=DR)
                    bias_sb = bpool.tile([PART, 512], F32, tag="bias")
                    nc.vector.tensor_copy(bias_sb[:], psb[:])
                    # 3) wd = w0 + (-w1) via DMA inline ALU (op(in,out) = in+out)
                    nc.gpsimd.dma_start(out=wd[:], in_=w0_d.ap()[pop, nbi],
                                        accum_op=mybir.AluOpType.add)
                    # 4) main pass: psum = x @ wd, evac with bias add
                    for m in range(mb):
                        ps = pspool.tile([PART, 512], F32)
                        msl = slice(m * PART, (m + 1) * PART)
                        for kd in range(nk):
                            ksl = slice(2 * kd, 2 * kd + 2)
                            nc.tensor.matmul(
                                ps[:], lhsT=xt[:, ksl, msl], rhs=wd[:, ksl, :],
                                start=(kd == 0), stop=(kd == nk - 1), perf_mode=DR)
                        ot = opool.tile([PART, 512], F32)
                        # out = psum - (-bias)
                        nc.vector.tensor_tensor(
                            ot[:], ps[:], bias_sb[:], mybir.AluOpType.subtract)
                        nc.scalar.dma_start(
                            out=out_d.ap()[pop, msl, nbi * 512:(nbi + 1) * 512],
                            in_=ot[:])
    nc.compile()
    return nc


def prep_core_inputs(x, w, core, ppc=PPC, negate_w1=False):
    """Layout-only host prep for one core: slice pops, transpose x, tile, cast.
    With negate_w1, the fp8 cast of w1 carries a sign flip (v2 sends -w1 so the
    device can form w0-w1 with the DMA ALU's accum add)."""
    p0 = core * ppc
    b, i_dim = x.shape[1], x.shape[2]
    o_dim = w.shape[4]
    kt = i_dim // PART
    nb = o_dim // 512
    xs = x[p0:p0 + ppc]                       # [ppc, B, I]
    # xT partition-tiled: [ppc, 128, kt, B];  xt[p, kp, kti, b] = x[p, b, kti*128+kp]
    xt = np.ascontiguousarray(
        xs.reshape(ppc, b, kt, PART).transpose(0, 3, 2, 1)
    ).astype(NP_FP8)
    ws = w[:, p0:p0 + ppc, 0]                 # [2, ppc, I, O]
    # [2, ppc, nb, 128, kt, 512]; wt[j,p,nbi,kp,kti,no] = w[j,p,kti*128+kp, nbi*512+no]
    wt = np.ascontiguousarray(
        ws.reshape(2, ppc, kt, PART, nb, 512).transpose(0, 1, 4, 3, 2, 5)
    )
    w0 = wt[0].astype(NP_FP8)
    w1 = (-wt[1]).astype(NP_FP8) if negate_w1 else wt[1].astype(NP_FP8)
    return {"xt": xt, "w0": w0, "w1": w1}


_NC_CACHE = {}

# which builder kernel() uses: 1 = concat (x@w0 + notx@w1), 2 = DMA-subtract trick
K_VERSION = int(os.environ.get("EVO_KERNEL_VERSION", "5"))


def _get_nc():
    if "nc" not in _NC_CACHE:
        builder = {1: build_nc, 2: build_nc_v2, 3: build_nc_v3,
                   4: build_nc_v4, 5: build_nc_v5}[K_VERSION]
        _NC_CACHE["nc"] = builder()
    return _NC_CACHE["nc"]


def kernel(x, w):
    x = np.asarray(x)
    w = np.asarray(w)
    nc = _get_nc()
    in_maps = [prep_core_inputs(x, w, c, negate_w1=(K_VERSION == 2))
               for c in range(N_CORES)]
    res = run_bass_kernel_spmd(nc, in_maps, list(range(N_CORES)))
    out = np.concatenate([res.results[c]["out"] for c in range(N_CORES)], axis=0)
    return np.ascontiguousarray(out.astype(np.float32))



# revision 7
# speedup vs baseline: 1.1273x; 1.1273x over previous
"""Bass/Trainium2 kernel for nn_EvoBinarizedLayer.

Reference computation (P=16 populations, B=512, I=O=2048, all values 0/1):
    out[p,b,o] = sum_i x[p,b,i]*w0[p,i,o] + (1-x[p,b,i])*w1[p,i,o]

Strategy:
  - Shard population dim P across 8 cores (2 pops/core), embarrassingly parallel.
  - Cast x/w to fp8e4m3 on host (0/1 values are exact); compute notx = 1-x on
    device (ACT/DVE); accumulate x@w0 + notx@w1 into the same PSUM bank via a
    single K=4096 "concat" contraction -> one accumulation group, no bias pass.
  - fp8 DoubleRow matmuls (K=256 per MM) for 2x PE throughput.
  - PSUM f32 accumulation of 0/1 products is exact (max 4096 < 2^24), so the
    result is bit-exact vs the f32 reference.

Host-side work is layout only: slicing, transpose, dtype cast, and the final
gather. All arithmetic (notx, matmuls) happens on device.
"""

import os

import numpy as np
import ml_dtypes

from concourse import bacc, tile, mybir
from concourse.bass_utils import run_bass_kernel_spmd

P_TOT, B, I, O = 16, 512, 2048, 2048
N_CORES = 8
PPC = P_TOT // N_CORES  # pops per core = 2
PART = 128

FP8 = mybir.dt.float8e4
F32 = mybir.dt.float32
NP_FP8 = ml_dtypes.float8_e4m3


def build_nc(ppc=PPC, b=B, i_dim=I, o_dim=O, n_cores=N_CORES, use_dr=True):
    """Build + compile the per-core Bass program (SPMD: same program, 8 cores)."""
    kt = i_dim // PART          # k-subtiles per weight tensor (16)
    nb = o_dim // 512           # o-blocks (4)
    mb = b // PART              # b-subtiles (4)
    DR = mybir.MatmulPerfMode.DoubleRow if use_dr else None
    kstep = 2 if use_dr else 1

    nc = bacc.Bacc("TRN2", target_bir_lowering=False, debug=False,
                   num_devices=n_cores)

    xt_d = nc.dram_tensor("xt", [ppc, PART, kt, b], FP8, kind="ExternalInput")
    w0_d = nc.dram_tensor("w0", [ppc, nb, PART, kt, 512], FP8, kind="ExternalInput")
    w1_d = nc.dram_tensor("w1", [ppc, nb, PART, kt, 512], FP8, kind="ExternalInput")
    out_d = nc.dram_tensor("out", [ppc, b, o_dim], F32, kind="ExternalOutput")

    with tile.TileContext(nc) as tc:
        with (
            tc.tile_pool(name="warm", bufs=1) as warm,
            tc.tile_pool(name="xpool", bufs=2) as xpool,
            tc.tile_pool(name="wpool", bufs=8) as wpool,
            tc.tile_pool(name="opool", bufs=4) as opool,
            tc.tile_pool(name="pspool", bufs=4, space="PSUM") as pspool,
            tc.tile_pool(name="warmps", bufs=1, space="PSUM") as warmps,
        ):
            for pop in range(ppc):
                xt = xpool.tile([PART, kt, b], FP8, tag="xt")
                nxt = xpool.tile([PART, kt, b], FP8, tag="nxt")
                # x chunked on the scalar ring ahead of w1: the first matmul
                # needs only xt[:, 0:2, :], so a 256KB first chunk unblocks
                # the first LDWEIGHTS ~10us sooner than one 1MB transfer.
                xch = min(4, kt)
                for ch in range(0, kt, xch):
                    nc.scalar.dma_start(out=xt[:, ch:ch + xch, :],
                                        in_=xt_d.ap()[pop, :, ch:ch + xch, :])
                    # notx = 1 - x  ==  (x * -1) + 1, per chunk
                    nc.vector.tensor_scalar(
                        nxt[:, ch:ch + xch, :], xt[:, ch:ch + xch, :], -1.0, 1.0,
                        mybir.AluOpType.mult, mybir.AluOpType.add,
                    )
                for nbi in range(nb):
                    w0t = wpool.tile([PART, kt, 512], FP8, tag="w")
                    w1t = wpool.tile([PART, kt, 512], FP8, tag="w")
                    # w0 loads on the sync HWDGE ring, w1 on the scalar HWDGE
                    # ring (output stores go via gpsimd/SWDGE) so stores never
                    # block weight prefetch in a shared FIFO. Chunked k-wise so
                    # the first matmuls start before the whole block lands; the
                    # very first block uses finer chunks to cut the startup
                    # bubble before the first LDWEIGHTS.
                    wch = 2 if (pop == 0 and nbi == 0) else 4
                    for ch in range(0, kt, wch):
                        nc.sync.dma_start(
                            out=w0t[:, ch:ch + wch, :],
                            in_=w0_d.ap()[pop, nbi, :, ch:ch + wch, :])
                        nc.scalar.dma_start(
                            out=w1t[:, ch:ch + wch, :],
                            in_=w1_d.ap()[pop, nbi, :, ch:ch + wch, :])
                    for m in range(mb):
                        ps = pspool.tile([PART, 512], F32)
                        msl = slice(m * PART, (m + 1) * PART)
                        nk = kt // kstep
                        for kd in range(nk):
                            ksl = slice(kd * kstep, (kd + 1) * kstep)
                            nc.tensor.matmul(
                                ps[:], lhsT=xt[:, ksl, msl], rhs=w0t[:, ksl, :],
                                start=(kd == 0), stop=False, perf_mode=DR,
                            )
                        for kd in range(nk):
                            ksl = slice(kd * kstep, (kd + 1) * kstep)
                            nc.tensor.matmul(
                                ps[:], lhsT=nxt[:, ksl, msl], rhs=w1t[:, ksl, :],
                                start=False, stop=(kd == nk - 1), perf_mode=DR,
                            )
                        ot = opool.tile([PART, 512], F32)
                        nc.vector.tensor_copy(ot[:], ps[:])
                        nc.gpsimd.dma_start(
                            out=out_d.ap()[pop, msl, nbi * 512:(nbi + 1) * 512],
                            in_=ot[:],
                        )
    nc.compile()
    return nc


def build_nc_v3(ppc=PPC, b=B, i_dim=I, o_dim=O, n_cores=N_CORES):
    """v3: concat scheme (as v1) with stationary reuse.

    All weights for one population stay SBUF-resident (8MB fp8); the matmul
    loop is m -> half -> kd -> nb so one LDWEIGHTS serves 4 matmuls (one per
    o-block), cutting LDW traffic 4x and keeping the PE stream dense. PSUM
    holds 4 accumulating banks (one per o-block) per m-subtile.
    """
    kt = i_dim // PART
    nb = o_dim // 512
    mb = b // PART
    DR = mybir.MatmulPerfMode.DoubleRow
    nk = kt // 2

    nc = bacc.Bacc("TRN2", target_bir_lowering=False, debug=False,
                   num_devices=n_cores)

    xt_d = nc.dram_tensor("xt", [ppc, PART, kt, b], FP8, kind="ExternalInput")
    w0_d = nc.dram_tensor("w0", [ppc, nb, PART, kt, 512], FP8, kind="ExternalInput")
    w1_d = nc.dram_tensor("w1", [ppc, nb, PART, kt, 512], FP8, kind="ExternalInput")
    out_d = nc.dram_tensor("out", [ppc, b, o_dim], F32, kind="ExternalOutput")

    with tile.TileContext(nc) as tc:
        with (
            tc.tile_pool(name="xpool", bufs=2) as xpool,
            tc.tile_pool(name="wpool", bufs=2 * nb * 2) as wpool,
            tc.tile_pool(name="opool", bufs=6) as opool,
            tc.tile_pool(name="pspool", bufs=8, space="PSUM") as pspool,
        ):
            for pop in range(ppc):
                xt = xpool.tile([PART, kt, b], FP8, tag="xt")
                nxt = xpool.tile([PART, kt, b], FP8, tag="nxt")
                nc.gpsimd.dma_start(out=xt[:], in_=xt_d.ap()[pop])
                nc.vector.tensor_scalar(
                    nxt[:], xt[:], -1.0, 1.0,
                    mybir.AluOpType.mult, mybir.AluOpType.add,
                )
                # all weights for this pop, k-chunked so matmuls start early;
                # w0 on the sync HWDGE ring, w1 on the scalar HWDGE ring
                w0t = [wpool.tile([PART, kt, 512], FP8, tag="w",
                                  name=f"w0t_{pop}_{i}") for i in range(nb)]
                w1t = [wpool.tile([PART, kt, 512], FP8, tag="w",
                                  name=f"w1t_{pop}_{i}") for i in range(nb)]
                for ch in range(0, kt, 4):
                    for nbi in range(nb):
                        nc.sync.dma_start(
                            out=w0t[nbi][:, ch:ch + 4, :],
                            in_=w0_d.ap()[pop, nbi, :, ch:ch + 4, :])
                        nc.scalar.dma_start(
                            out=w1t[nbi][:, ch:ch + 4, :],
                            in_=w1_d.ap()[pop, nbi, :, ch:ch + 4, :])
                for m in range(mb):
                    msl = slice(m * PART, (m + 1) * PART)
                    pss = [pspool.tile([PART, 512], F32, tag="ps",
                                       name=f"ps_{pop}_{m}_{i}") for i in range(nb)]
                    for half, (xsrc, wt) in enumerate(((xt, w0t), (nxt, w1t))):
                        for kd in range(nk):
                            ksl = slice(2 * kd, 2 * kd + 2)
                            for nbi in range(nb):
                                nc.tensor.matmul(
                                    pss[nbi][:], lhsT=xsrc[:, ksl, msl],
                                    rhs=wt[nbi][:, ksl, :],
                                    start=(half == 0 and kd == 0),
                                    stop=(half == 1 and kd == nk - 1),
                                    perf_mode=DR,
                                )
                    for nbi in range(nb):
                        ot = opool.tile([PART, 512], F32)
                        nc.vector.tensor_copy(ot[:], pss[nbi][:])
                        nc.gpsimd.dma_start(
                            out=out_d.ap()[pop, msl, nbi * 512:(nbi + 1) * 512],
                            in_=ot[:],
                        )
    nc.compile()
    return nc


def build_nc_v4(ppc=PPC, b=B, i_dim=I, o_dim=O, n_cores=N_CORES):
    """v4: out = x@(w0-w1) + colsum(w1), wd built by DVE+gpsimd tensor_tensor.

    Halves the PE matmul stream vs the concat scheme (K=2048 instead of 4096).
    Per o-block: load w0/w1, bias = colsum(w1) via an all-ones DR matmul,
    wd = w0-w1 with the k-subtiles split between vector (11) and gpsimd (5)
    engines, main matmuls accumulate x@wd, and the DVE evacuation adds bias
    (tensor_tensor add against a bias tile copied from the bias PSUM bank).
    """
    kt = i_dim // PART
    nb = o_dim // 512
    mb = b // PART
    DR = mybir.MatmulPerfMode.DoubleRow
    nk = kt // 2
    # all subtract work on DVE: offloading 2 k-subtiles to gpsimd measured
    # 128.6us vs 128.0us all-DVE — the DVE's 23us of idle means it is not
    # strictly binding, and the gpsimd offload does not pay
    kdve = kt

    nc = bacc.Bacc("TRN2", target_bir_lowering=False, debug=False,
                   num_devices=n_cores)

    xt_d = nc.dram_tensor("xt", [ppc, PART, kt, b], FP8, kind="ExternalInput")
    w0_d = nc.dram_tensor("w0", [ppc, nb, PART, kt, 512], FP8, kind="ExternalInput")
    w1_d = nc.dram_tensor("w1", [ppc, nb, PART, kt, 512], FP8, kind="ExternalInput")
    out_d = nc.dram_tensor("out", [ppc, b, o_dim], F32, kind="ExternalOutput")

    with tile.TileContext(nc) as tc:
        with (
            tc.tile_pool(name="const", bufs=1) as const,
            tc.tile_pool(name="xpool", bufs=2) as xpool,
            tc.tile_pool(name="wsrc", bufs=6) as wsrc,
            tc.tile_pool(name="wdpool", bufs=4) as wdpool,
            tc.tile_pool(name="bpool", bufs=3) as bpool,
            tc.tile_pool(name="opool", bufs=4) as opool,
            tc.tile_pool(name="pspool", bufs=4, space="PSUM") as pspool,
            tc.tile_pool(name="psbias", bufs=2, space="PSUM") as psbias,
        ):
            ones = const.tile([PART, 2, PART], FP8)
            nc.vector.memset(ones[:], 1.0)
            xts = {}
            state = {}
            blocks = [(pop, nbi) for pop in range(ppc) for nbi in range(nb)]

            def prepare(pop, nbi):
                if nbi == 0:
                    xt = xpool.tile([PART, kt, b], FP8, tag="xt",
                                    name=f"xt_{pop}")
                    xch = min(4, kt)
                    for ch in range(0, kt, xch):
                        nc.scalar.dma_start(
                            out=xt[:, ch:ch + xch, :],
                            in_=xt_d.ap()[pop, :, ch:ch + xch, :])
                    xts[pop] = xt
                w0t = wsrc.tile([PART, kt, 512], FP8, tag="ws",
                                name=f"w0t_{pop}_{nbi}")
                w1t = wsrc.tile([PART, kt, 512], FP8, tag="ws",
                                name=f"w1t_{pop}_{nbi}")
                wch = 2 if (pop == 0 and nbi == 0) else 4
                for ch in range(0, kt, wch):
                    nc.sync.dma_start(
                        out=w1t[:, ch:ch + wch, :],
                        in_=w1_d.ap()[pop, nbi, :, ch:ch + wch, :])
                    nc.scalar.dma_start(
                        out=w0t[:, ch:ch + wch, :],
                        in_=w0_d.ap()[pop, nbi, :, ch:ch + wch, :])
                # bias = colsum(w1) (all rows of psb identical)
                psb = psbias.tile([PART, 512], F32, tag="psb")
                for kd in range(nk):
                    ksl = slice(2 * kd, 2 * kd + 2)
                    nc.tensor.matmul(
                        psb[:], lhsT=ones[:], rhs=w1t[:, ksl, :],
                        start=(kd == 0), stop=(kd == nk - 1), perf_mode=DR)
                bias_sb = bpool.tile([PART, 512], F32, tag="bias")
                nc.vector.tensor_copy(bias_sb[:], psb[:])
                # wd = w0 - w1 on DVE in fine k-chunks; emitted one block
                # AHEAD of the consuming matmuls (software pipeline) so these
                # sit before the previous block's evacuations in the DVE FIFO
                wd = wdpool.tile([PART, kt, 512], FP8, tag="wd")
                sch = max(1, kt // 8)
                for ch in range(0, kdve, sch):
                    nc.vector.tensor_tensor(
                        wd[:, ch:ch + sch, :], w0t[:, ch:ch + sch, :],
                        w1t[:, ch:ch + sch, :], mybir.AluOpType.subtract)
                if kdve < kt:
                    nc.gpsimd.tensor_tensor(
                        wd[:, kdve:, :], w0t[:, kdve:, :], w1t[:, kdve:, :],
                        mybir.AluOpType.subtract)
                state[(pop, nbi)] = (wd, bias_sb)

            def main(pop, nbi):
                wd, bias_sb = state.pop((pop, nbi))
                xt = xts[pop]
                for m in range(mb):
                    ps = pspool.tile([PART, 512], F32, tag="ps",
                                     name=f"ps_{pop}_{nbi}_{m}")
                    msl = slice(m * PART, (m + 1) * PART)
                    for kd in range(nk):
                        ksl = slice(2 * kd, 2 * kd + 2)
                        nc.tensor.matmul(
                            ps[:], lhsT=xt[:, ksl, msl], rhs=wd[:, ksl, :],
                            start=(kd == 0), stop=(kd == nk - 1), perf_mode=DR)
                    ot = opool.tile([PART, 512], F32, tag="ot",
                                    name=f"ot_{pop}_{nbi}_{m}")
                    nc.vector.tensor_tensor(
                        ot[:], ps[:], bias_sb[:], mybir.AluOpType.add)
                    nc.gpsimd.dma_start(
                        out=out_d.ap()[pop, msl, nbi * 512:(nbi + 1) * 512],
                        in_=ot[:])

            for i in range(len(blocks) + 1):
                if i < len(blocks):
                    prepare(*blocks[i])
                if i > 0:
                    main(*blocks[i - 1])
    nc.compile()
    return nc


def build_nc_v5(ppc=PPC, b=B, i_dim=I, o_dim=O, n_cores=N_CORES,
                kdve=6, sub_ch=4):
    """v5: v4 algebra (out = x@(w0-w1) + colsum(w1)) rebalanced across engines.

    Baseline v4 bottleneck was the DVE: 99.7us busy (wd subtract 78us + evac
    22us) vs PE 69us warm floor. Changes:
      - wd subtract split DVE (k-subtiles [0:kdve)) / gpsimd ([kdve:kt)) —
        Pool runs 1.2GHz vs DVE 0.96GHz.
      - PSUM evac (ps + bias) split DVE (m=0,2) / gpsimd (m=1,3), emitting
        f16 directly (exact: outputs are integers <= 2048 = 2^11).
      - bias psum->SBUF copies on the ACT engine (otherwise idle).
      - stores on the sync HWDGE ring (f16, half the bytes); w0+x on scalar,
        w1 on sync; gpsimd dispatches no DMA.
      - software pipeline: loads 2 blocks ahead, bias+subtract 1 block ahead
        of the main matmul stream so the PE never stalls and HAM stays warm;
        dummy warm-up matmuls cover the initial DMA latency.
    """
    kt = i_dim // PART
    nb = o_dim // 512
    mb = b // PART
    DR = mybir.MatmulPerfMode.DoubleRow
    nk = kt // 2
    F16 = mybir.dt.float16

    nc = bacc.Bacc("TRN2", target_bir_lowering=False, debug=False,
                   num_devices=n_cores)

    xt_d = nc.dram_tensor("xt", [ppc, PART, kt, b], FP8, kind="ExternalInput")
    w0_d = nc.dram_tensor("w0", [ppc, nb, PART, kt, 512], FP8, kind="ExternalInput")
    w1_d = nc.dram_tensor("w1", [ppc, nb, PART, kt, 512], FP8, kind="ExternalInput")
    out_d = nc.dram_tensor("out", [ppc, b, o_dim], F16, kind="ExternalOutput")

    with tile.TileContext(nc) as tc:
        with (
            tc.tile_pool(name="const", bufs=1) as const,
            tc.tile_pool(name="xpool", bufs=2) as xpool,
            tc.tile_pool(name="wsrc", bufs=6) as wsrc,
            tc.tile_pool(name="wdpool", bufs=3) as wdpool,
            tc.tile_pool(name="bpool", bufs=3) as bpool,
            tc.tile_pool(name="opool", bufs=8) as opool,
            tc.tile_pool(name="pspool", bufs=4, space="PSUM") as pspool,
            tc.tile_pool(name="psbias", bufs=2, space="PSUM") as psbias,
            tc.tile_pool(name="pswarm", bufs=1, space="PSUM") as pswarm,
        ):
            ones = const.tile([PART, 2, PART], FP8)
            nc.vector.memset(ones[:], 1.0)
            # HAM warm-up: ~8 N=512 matmuls on garbage keep the PE busy
            # through the cold window while the first weight DMAs land.
            warm = pswarm.tile([PART, 512], F32)
            wsrc_warm = const.tile([PART, 2, 512], FP8)
            nc.vector.memset(wsrc_warm[:], 1.0)
            for wi in range(8):
                nc.tensor.matmul(warm[:], lhsT=ones[:], rhs=wsrc_warm[:],
                                 start=(wi == 0), stop=(wi == 7), perf_mode=DR)

            xts = {}
            w0ts = {}
            w1ts = {}
            wds = {}
            biases = {}
            blocks = [(pop, nbi) for pop in range(ppc) for nbi in range(nb)]

            def load(i):
                pop, nbi = blocks[i]
                if nbi == 0:
                    xt = xpool.tile([PART, kt, b], FP8, tag="xt",
                                    name=f"xt_{pop}")
                    for ch in range(0, kt, 8):
                        nc.scalar.dma_start(
                            out=xt[:, ch:ch + 8, :],
                            in_=xt_d.ap()[pop, :, ch:ch + 8, :])
                    xts[pop] = xt
                w0t = wsrc.tile([PART, kt, 512], FP8, tag="ws",
                                name=f"w0t_{i}")
                w1t = wsrc.tile([PART, kt, 512], FP8, tag="ws",
                                name=f"w1t_{i}")
                wch = 2 if i == 0 else 4
                for ch in range(0, kt, wch):
                    nc.sync.dma_start(
                        out=w1t[:, ch:ch + wch, :],
                        in_=w1_d.ap()[pop, nbi, :, ch:ch + wch, :])
                    nc.scalar.dma_start(
                        out=w0t[:, ch:ch + wch, :],
                        in_=w0_d.ap()[pop, nbi, :, ch:ch + wch, :])
                w0ts[i], w1ts[i] = w0t, w1t

            def prep(i):
                w0t, w1t = w0ts[i], w1ts.pop(i)
                # bias = colsum(w1) via all-ones DR matmul chain
                psb = psbias.tile([PART, 512], F32, tag="psb")
                for kd in range(nk):
                    ksl = slice(2 * kd, 2 * kd + 2)
                    nc.tensor.matmul(
                        psb[:], lhsT=ones[:], rhs=w1t[:, ksl, :],
                        start=(kd == 0), stop=(kd == nk - 1), perf_mode=DR)
                bias_sb = bpool.tile([PART, 512], F32, tag="bias")
                nc.scalar.activation(bias_sb[:], psb[:],
                                     mybir.ActivationFunctionType.Copy)
                biases[i] = bias_sb
                # wd = w0 - w1, split DVE / gpsimd
                wd = wdpool.tile([PART, kt, 512], FP8, tag="wd")
                for ch in range(0, kdve, sub_ch):
                    ce = min(ch + sub_ch, kdve)
                    nc.vector.tensor_tensor(
                        wd[:, ch:ce, :], w0t[:, ch:ce, :],
                        w1t[:, ch:ce, :], mybir.AluOpType.subtract)
                for ch in range(kdve, kt, sub_ch):
                    ce = min(ch + sub_ch, kt)
                    nc.gpsimd.tensor_tensor(
                        wd[:, ch:ce, :], w0t[:, ch:ce, :],
                        w1t[:, ch:ce, :], mybir.AluOpType.subtract)
                wds[i] = wd
                w0ts.pop(i)

            def main(i):
                pop, nbi = blocks[i]
                wd = wds.pop(i)
                bias_sb = biases.pop(i)
                xt = xts[pop]
                for m in range(mb):
                    ps = pspool.tile([PART, 512], F32, tag="ps",
                                     name=f"ps_{i}_{m}")
                    msl = slice(m * PART, (m + 1) * PART)
                    for kd in range(nk):
                        ksl = slice(2 * kd, 2 * kd + 2)
                        nc.tensor.matmul(
                            ps[:], lhsT=xt[:, ksl, msl], rhs=wd[:, ksl, :],
                            start=(kd == 0), stop=(kd == nk - 1), perf_mode=DR)
                    ot = opool.tile([PART, 512], F16, tag="ot",
                                    name=f"ot_{i}_{m}")
                    # gpsimd cannot read PSUM -> evac is DVE-only
                    nc.vector.tensor_tensor(
                        ot[:], ps[:], bias_sb[:], mybir.AluOpType.add)
                    nc.sync.dma_start(
                        out=out_d.ap()[pop, msl, nbi * 512:(nbi + 1) * 512],
                        in_=ot[:])

            load(0)
            load(1)
            prep(0)
            for i in range(len(blocks)):
                if i + 2 < len(blocks):
                    load(i + 2)
                if i + 1 < len(blocks):
                    prep(i + 1)
                main(i)
    nc.compile()
    return nc


def build_nc_v6(ppc=PPC, b=B, i_dim=I, o_dim=O, n_cores=N_CORES,
                kdve=11, fold=4):
    """v6: deep pipeline + bias folded into PSUM so evac moves to ACT.

    Measured engine rates (v5 trace): DVE subtract 0.61us/k-subtile, gpsimd
    1.23us/k-subtile (Pool tensor_tensor is ~2.3ns/elem, not the modeled
    0.83), ACT copy ~0.7us, warm DR matmul 216ns.

    Per block (8 total): PE = 8 bias-chain MMs + 32 main DR MMs + `fold`
    K=1 bf16 bias MMs; DVE = subtract of k-subtiles [0:kdve); gpsimd =
    subtract [kdve:kt); ACT = `fold` evac copies + (4-fold) on DVE as
    tensor_tensor bias-adds + bias psum->SBUF copies + w0/x DMA dispatch;
    sync = w1 loads + f16 stores.

    Pipeline: loads run 4 blocks ahead, bias-chain+subtract 2 ahead, so the
    PE stream (the 69-76us floor) never waits on DMA or the vector engines.
    """
    kt = i_dim // PART
    nb = o_dim // 512
    mb = b // PART
    DR = mybir.MatmulPerfMode.DoubleRow
    nk = kt // 2
    F16 = mybir.dt.float16
    BF16 = mybir.dt.bfloat16

    nc = bacc.Bacc("TRN2", target_bir_lowering=False, debug=False,
                   num_devices=n_cores)

    xt_d = nc.dram_tensor("xt", [ppc, PART, kt, b], FP8, kind="ExternalInput")
    w0_d = nc.dram_tensor("w0", [ppc, nb, PART, kt, 512], FP8, kind="ExternalInput")
    w1_d = nc.dram_tensor("w1", [ppc, nb, PART, kt, 512], FP8, kind="ExternalInput")
    out_d = nc.dram_tensor("out", [ppc, b, o_dim], F16, kind="ExternalOutput")

    with tile.TileContext(nc) as tc:
        with (
            tc.tile_pool(name="const", bufs=1) as const,
            tc.tile_pool(name="xpool", bufs=2) as xpool,
            tc.tile_pool(name="wsrc", bufs=8) as wsrc,
            tc.tile_pool(name="wdpool", bufs=3) as wdpool,
            tc.tile_pool(name="bpool", bufs=3) as bpool,
            tc.tile_pool(name="opool", bufs=8) as opool,
            tc.tile_pool(name="pspool", bufs=6, space="PSUM") as pspool,
            tc.tile_pool(name="psbias", bufs=2, space="PSUM") as psbias,
        ):
            ones = const.tile([PART, 2, PART], FP8)
            nc.vector.memset(ones[:], 1.0)
            ones1 = const.tile([1, PART], BF16)
            nc.vector.memset(ones1[:], 1.0)
            # HAM warm-up while the first DMAs land
            warm = pspool.tile([PART, 512], F32, tag="ps", name="warm")
            warmsrc = const.tile([PART, 2, 512], FP8)
            nc.vector.memset(warmsrc[:], 1.0)
            for wi in range(8):
                nc.tensor.matmul(warm[:], lhsT=ones[:], rhs=warmsrc[:],
                                 start=(wi == 0), stop=(wi == 7), perf_mode=DR)

            xts = {}
            w0ts = {}
            w1ts = {}
            wds = {}
            biases = {}
            blocks = [(pop, nbi) for pop in range(ppc) for nbi in range(nb)]
            NBLK = len(blocks)

            def load(i):
                pop, nbi = blocks[i]
                if nbi == 0:
                    xt = xpool.tile([PART, kt, b], FP8, tag="xt",
                                    name=f"xt_{pop}")
                    for ch in range(0, kt, 8):
                        nc.scalar.dma_start(
                            out=xt[:, ch:ch + 8, :],
                            in_=xt_d.ap()[pop, :, ch:ch + 8, :])
                    xts[pop] = xt
                w0t = wsrc.tile([PART, kt, 512], FP8, tag="ws",
                                name=f"w0t_{i}")
                w1t = wsrc.tile([PART, kt, 512], FP8, tag="ws",
                                name=f"w1t_{i}")
                wch = 2 if i == 0 else 4
                for ch in range(0, kt, wch):
                    nc.sync.dma_start(
                        out=w1t[:, ch:ch + wch, :],
                        in_=w1_d.ap()[pop, nbi, :, ch:ch + wch, :])
                    nc.scalar.dma_start(
                        out=w0t[:, ch:ch + wch, :],
                        in_=w0_d.ap()[pop, nbi, :, ch:ch + wch, :])
                w0ts[i], w1ts[i] = w0t, w1t

            def prep(i):
                w0t, w1t = w0ts.pop(i), w1ts.pop(i)
                # bias = colsum(w1) via all-ones DR matmul chain (PE)
                psb = psbias.tile([PART, 512], F32, tag="psb")
                for kd in range(nk):
                    ksl = slice(2 * kd, 2 * kd + 2)
                    nc.tensor.matmul(
                        psb[:], lhsT=ones[:], rhs=w1t[:, ksl, :],
                        start=(kd == 0), stop=(kd == nk - 1), perf_mode=DR)
                bias_bf = bpool.tile([1, 512], BF16, tag="biasbf")
                nc.scalar.activation(bias_bf[:], psb[0:1, :],
                                     mybir.ActivationFunctionType.Copy)
                bias_sb = None
                if fold < mb:
                    bias_sb = bpool.tile([PART, 512], F32, tag="bias")
                    nc.scalar.activation(bias_sb[:], psb[:],
                                         mybir.ActivationFunctionType.Copy)
                biases[i] = (bias_bf, bias_sb)
                # wd = w0 - w1: one big DVE op + one big gpsimd op
                wd = wdpool.tile([PART, kt, 512], FP8, tag="wd")
                if kdve > 0:
                    nc.vector.tensor_tensor(
                        wd[:, 0:kdve, :], w0t[:, 0:kdve, :],
                        w1t[:, 0:kdve, :], mybir.AluOpType.subtract)
                if kdve < kt:
                    nc.gpsimd.tensor_tensor(
                        wd[:, kdve:, :], w0t[:, kdve:, :],
                        w1t[:, kdve:, :], mybir.AluOpType.subtract)
                wds[i] = wd

            def main(i):
                pop, nbi = blocks[i]
                wd = wds.pop(i)
                bias_bf, bias_sb = biases.pop(i)
                xt = xts[pop]
                for m in range(mb):
                    ps = pspool.tile([PART, 512], F32, tag="ps",
                                     name=f"ps_{i}_{m}")
                    msl = slice(m * PART, (m + 1) * PART)
                    folded = m < fold
                    for kd in range(nk):
                        ksl = slice(2 * kd, 2 * kd + 2)
                        nc.tensor.matmul(
                            ps[:], lhsT=xt[:, ksl, msl], rhs=wd[:, ksl, :],
                            start=(kd == 0),
                            stop=(kd == nk - 1 and not folded), perf_mode=DR)
                    ot = opool.tile([PART, 512], F16, tag="ot",
                                    name=f"ot_{i}_{m}")
                    if folded:
                        # += bias via K=1 bf16 matmul, then pure-copy evac on ACT
                        nc.tensor.matmul(ps[:], lhsT=ones1[:], rhs=bias_bf[:],
                                         start=False, stop=True)
                        nc.scalar.activation(ot[:], ps[:],
                                             mybir.ActivationFunctionType.Copy)
                    else:
                        nc.vector.tensor_tensor(
                            ot[:], ps[:], bias_sb[:], mybir.AluOpType.add)
                    nc.sync.dma_start(
                        out=out_d.ap()[pop, msl, nbi * 512:(nbi + 1) * 512],
                        in_=ot[:])

            for i in range(min(4, NBLK)):
                load(i)
            prep(0)
            prep(1)
            for i in range(NBLK):
                if i + 4 < NBLK:
                    load(i + 4)
                main(i)
                if i + 2 < NBLK:
                    prep(i + 2)
    nc.compile()
    return nc


def build_nc_v2(ppc=PPC, b=B, i_dim=I, o_dim=O, n_cores=N_CORES):
    """v2: algebraic rewrite out = x@(w0-w1) + colsum(w1).

    The w1 input tensor holds -w1 (sign applied during the host fp8 cast;
    walrus rejects cce_op=subtract but accepts add):
    - wd = w0 + (-w1) computed by the gpsimd DMA inline ALU (accum_op=add)
      while loading w0 — zero compute-engine cost.
    - colsum(-w1) = -bias via an all-ones stationary matmul against the tile
      while it still holds -w1, once per o-block.
    - main pass: psum = x @ wd, half the PE work of v1; evacuated as
      psum - (-bias) with a DVE tensor_tensor subtract.
    All values stay exact: x in {0,1}, wd in {-1,0,1} (fp8 exact), bias and
    accumulation in f32 (integers < 2^24).
    """
    kt = i_dim // PART
    nb = o_dim // 512
    mb = b // PART
    DR = mybir.MatmulPerfMode.DoubleRow
    nk = kt // 2

    nc = bacc.Bacc("TRN2", target_bir_lowering=False, debug=False,
                   num_devices=n_cores)

    xt_d = nc.dram_tensor("xt", [ppc, PART, kt, b], FP8, kind="ExternalInput")
    w0_d = nc.dram_tensor("w0", [ppc, nb, PART, kt, 512], FP8, kind="ExternalInput")
    w1_d = nc.dram_tensor("w1", [ppc, nb, PART, kt, 512], FP8, kind="ExternalInput")
    out_d = nc.dram_tensor("out", [ppc, b, o_dim], F32, kind="ExternalOutput")

    with tile.TileContext(nc) as tc:
        with (
            tc.tile_pool(name="const", bufs=1) as const,
            tc.tile_pool(name="xpool", bufs=2) as xpool,
            tc.tile_pool(name="wpool", bufs=4) as wpool,
            tc.tile_pool(name="bpool", bufs=2) as bpool,
            tc.tile_pool(name="opool", bufs=4) as opool,
            tc.tile_pool(name="pspool", bufs=4, space="PSUM") as pspool,
            tc.tile_pool(name="psbias", bufs=2, space="PSUM") as psbias,
        ):
            ones = const.tile([PART, 2, PART], FP8)
            nc.vector.memset(ones[:], 1.0)
            for pop in range(ppc):
                xt = xpool.tile([PART, kt, b], FP8, tag="xt")
                nc.scalar.dma_start(out=xt[:], in_=xt_d.ap()[pop])
                for nbi in range(nb):
                    # 544-wide rows (512 data + 32 pad): keeps every SBUF write
                    # run at 512B so the accum DMA's RMW ucode accepts it (runs
                    # >512B crash the exec unit), and stops the AP optimizer
                    # from merging rows into one big run.
                    wdp = wpool.tile([PART, kt, 544], FP8, tag="w")
                    wd = wdp[:, :, :512]
                    # 1) load -w1 (sync HWDGE ring)
                    wch = min(8, kt)
                    for ch in range(0, kt, wch):
                        nc.sync.dma_start(
                            out=wd[:, ch:ch + wch, :],
                            in_=w1_d.ap()[pop, nbi, :, ch:ch + wch, :])
                    # 2) -bias = colsum(-w1) while the tile still holds -w1
                    psb = psbias.tile([PART, 512], F32)
                    for kd in range(nk):
                        ksl = slice(2 * kd, 2 * kd + 2)
                        nc.tensor.matmul(
                            psb[:], lhsT=ones[:], rhs=wd[:, ksl, :],
                            start=(kd == 0), stop=(kd == nk - 1), perf_mode=DR)
                    bias_sb = bpool.tile([PART, 512], F32, tag="bias")
                    nc.vector.tensor_copy(bias_sb[:], psb[:])
                    # 3) wd = w0 + (-w1) via DMA inline ALU (op(in,out) = in+out)
                    nc.gpsimd.dma_start(out=wd[:], in_=w0_d.ap()[pop, nbi],
                                        accum_op=mybir.AluOpType.add)
                    # 4) main pass: psum = x @ wd, evac with bias add
                    for m in range(mb):
                        ps = pspool.tile([PART, 512], F32)
                        msl = slice(m * PART, (m + 1) * PART)
                        for kd in range(nk):
                            ksl = slice(2 * kd, 2 * kd + 2)
                            nc.tensor.matmul(
                                ps[:], lhsT=xt[:, ksl, msl], rhs=wd[:, ksl, :],
                                start=(kd == 0), stop=(kd == nk - 1), perf_mode=DR)
                        ot = opool.tile([PART, 512], F32)
                        # out = psum - (-bias)
                        nc.vector.tensor_tensor(
                            ot[:], ps[:], bias_sb[:], mybir.AluOpType.subtract)
                        nc.scalar.dma_start(
                            out=out_d.ap()[pop, msl, nbi * 512:(nbi + 1) * 512],
                            in_=ot[:])
    nc.compile()
    return nc


def prep_core_inputs(x, w, core, ppc=PPC, negate_w1=False):
    """Layout-only host prep for one core: slice pops, transpose x, tile, cast.
    With negate_w1, the fp8 cast of w1 carries a sign flip (v2 sends -w1 so the
    device can form w0-w1 with the DMA ALU's accum add)."""
    p0 = core * ppc
    b, i_dim = x.shape[1], x.shape[2]
    o_dim = w.shape[4]
    kt = i_dim // PART
    nb = o_dim // 512
    xs = x[p0:p0 + ppc]                       # [ppc, B, I]
    # xT partition-tiled: [ppc, 128, kt, B];  xt[p, kp, kti, b] = x[p, b, kti*128+kp]
    xt = np.ascontiguousarray(
        xs.reshape(ppc, b, kt, PART).transpose(0, 3, 2, 1)
    ).astype(NP_FP8)
    ws = w[:, p0:p0 + ppc, 0]                 # [2, ppc, I, O]
    # [2, ppc, nb, 128, kt, 512]; wt[j,p,nbi,kp,kti,no] = w[j,p,kti*128+kp, nbi*512+no]
    wt = np.ascontiguousarray(
        ws.reshape(2, ppc, kt, PART, nb, 512).transpose(0, 1, 4, 3, 2, 5)
    )
    w0 = wt[0].astype(NP_FP8)
    w1 = (-wt[1]).astype(NP_FP8) if negate_w1 else wt[1].astype(NP_FP8)
    return {"xt": xt, "w0": w0, "w1": w1}


_NC_CACHE = {}

# which builder kernel() uses: 1 = concat (x@w0 + notx@w1), 2 = DMA-subtract trick
K_VERSION = int(os.environ.get("EVO_KERNEL_VERSION", "6"))
V6_KDVE = int(os.environ.get("EVO_KDVE", "11"))
V6_FOLD = int(os.environ.get("EVO_FOLD", "4"))


def _get_nc():
    if "nc" not in _NC_CACHE:
        if K_VERSION == 6:
            _NC_CACHE["nc"] = build_nc_v6(kdve=V6_KDVE, fold=V6_FOLD)
        else:
            builder = {1: build_nc, 2: build_nc_v2, 3: build_nc_v3,
                       4: build_nc_v4, 5: build_nc_v5}[K_VERSION]
            _NC_CACHE["nc"] = builder()
    return _NC_CACHE["nc"]


def kernel(x, w):
    x = np.asarray(x)
    w = np.asarray(w)
    nc = _get_nc()
    in_maps = [prep_core_inputs(x, w, c, negate_w1=(K_VERSION == 2))
               for c in range(N_CORES)]
    res = run_bass_kernel_spmd(nc, in_maps, list(range(N_CORES)))
    out = np.concatenate([res.results[c]["out"] for c in range(N_CORES)], axis=0)
    return np.ascontiguousarray(out.astype(np.float32))

